# revision 29
# baseline (speedup 1.0000x reference)
"""DGCNN forward kernel for Trainium2, data-parallel over batch on 8 cores.

Reference computation (per batch element, all in fp32):
  4 x [ kNN(k=20) -> edge features [x_j - x_i; x_i] -> 1x1 conv -> BatchNorm
        (training stats over the WHOLE batch) -> LeakyReLU(0.01) -> max over k ]
  then out = max_k(Wskip @ concat(x1_,x2_,x3_,x4_)).

Key device-side structure (one point cloud per core):
  - pairwise-distance matrix computed on the PE via an augmented matmul
    [f; ones]^T @ [f; -xx/2], scaled/biased on ACT into pd = 2G - xx_r - xx_j
  - top-20 via DVE max8/max_index/match_replace (3 rounds, jax tie semantics)
  - 1x1 conv commutes with the gather: y = (W_l@f)[idx] - ((W_l - W_r)@f)
    so only (64, N) matmuls run on PE and the gather runs on GPSIMD ap_gather
  - BN statistics: per-core sum/sumsq, AllReduce over the 8 cores, normalize
  - LeakyReLU(a) = (1+a)/2 * u + (1-a)/2 * |u|  (ACT Identity + Abs, GPSIMD add)
  - skip-path partial sums accumulate in DRAM; layer 4 fuses the final add
    with max-over-k and writes the output
"""

import numpy as np

import concourse.bass as bass
import concourse.bacc as bacc
import concourse.mybir as mybir
import concourse.tile as tile
from concourse.bass_utils import run_bass_kernel_spmd

F32 = mybir.dt.float32
U16 = mybir.dt.uint16
I16 = mybir.dt.int16
AF = mybir.ActivationFunctionType
ALU = mybir.AluOpType

B = 8
KNN = 20
EPS = 1e-5
ALPHA = 0.01


def build_program(n=2048, stage=99):
    assert n % 128 == 0
    nblk = n // 128          # 128-point row blocks
    nk2 = n * KNN // 2       # free size of the gathered tensor (both halves)
    # zg free layout is kk-outer: col = kk*(n/2) + (u*nblk+b)*16 + v
    cw = n                   # chunk width = 2 kk-groups
    nch = nk2 // cw          # = KNN // 2
    ct = min(512, n)          # matmul moving-dim tile
    cnt = float(B * n * KNN)  # BN population size (global over cores)

    nc = bacc.Bacc("TRN2", target_bir_lowering=False, debug=False, num_devices=B)
    groups = [list(range(B))]

    xT = nc.declare_dram_parameter("xT", [3, n], F32, isOutput=False)
    xaugl1 = nc.declare_dram_parameter("xaugl1", [5, n], F32, isOutput=False)
    xaugr1 = nc.declare_dram_parameter("xaugr1", [5, n], F32, isOutput=False)
    onesd = nc.declare_dram_parameter("onesd", [1, n], F32, isOutput=False)
    lw = [None] * 5   # [W_left.T | W_left.T]           (c, 128)
    rwn = [None] * 5  # [0 | -W_right.T]                (c, 128)
    ws = [None] * 5   # [Wskip_L.T | 0], [0 | Wskip_L.T] (64, 128) x 2
    wsb = [None] * 5
    for L, c in ((1, 3), (2, 64), (3, 64), (4, 64)):
        lw[L] = nc.declare_dram_parameter(f"lw{L}", [c, 128], F32, isOutput=False)
        rwn[L] = nc.declare_dram_parameter(f"rwn{L}", [c, 128], F32, isOutput=False)
        ws[L] = nc.declare_dram_parameter(f"wsa{L}", [64, 128], F32, isOutput=False)
        wsb[L] = nc.declare_dram_parameter(f"wsb{L}", [64, 128], F32, isOutput=False)
    gsT = nc.declare_dram_parameter("gsT", [64, 4], F32, isOutput=False)
    bsT = nc.declare_dram_parameter("bsT", [64, 4], F32, isOutput=False)
    out_e = nc.declare_dram_parameter("out", [64, n], F32, isOutput=True)

    with tile.TileContext(nc) as tc:
        with (
            tc.tile_pool(name="big", bufs=1) as big,
            tc.tile_pool(name="scratch", bufs=2) as sp,
            tc.tile_pool(name="small", bufs=2) as sm,
            tc.tile_pool(name="wpool", bufs=1) as wp,
            tc.tile_pool(name="pdps", bufs=3, space="PSUM") as pdps,
            tc.tile_pool(name="mmps", bufs=2, space="PSUM") as mmps,
            tc.tile_pool(name="ysps", bufs=2, space="PSUM") as ysps,
            tc.tile_pool(name="dram", bufs=1, space="DRAM") as dp,
        ):
            # ---- persistent SBUF tensors ----
            zg = big.tile([128, nk2], F32)            # gather out -> y -> xL_
            zrep = big.tile([128, n], F32)            # z replicated on both halves
            wrep = big.tile([128, n // 2], F32)       # (z - yc), wcol layout, both halves
            aug_l = big.tile([66, n], F32)            # [f; ones; -xx/2]
            aug_r = big.tile([66, n], F32)            # [f; -xx/2; ones]
            aug_l1 = big.tile([5, n], F32)            # layer-1 variants
            aug_r1 = big.tile([5, n], F32)
            idx_all = big.tile([128, nblk * 24], U16)  # raw top-24 per block
            idxs = big.tile([128, nk2 // 16], U16)     # wrapped gather table
            out_run = big.tile([128, n // 2], F32)     # final max accumulator
            acc_dram = dp.tile([128, nk2], F32)

            # ---- weights / constants ----
            lw_sb = [None] * 5
            rwn_sb = [None] * 5
            ws_sb = [None] * 5
            wsb_sb = [None] * 5
            for L, c in ((1, 3), (2, 64), (3, 64), (4, 64)):
                lw_sb[L] = wp.tile([c, 128], F32, name=f"lw_sb{L}")
                nc.sync.dma_start(lw_sb[L][:, :], lw[L][:, :])
                rwn_sb[L] = wp.tile([c, 128], F32, name=f"rwn_sb{L}")
                nc.sync.dma_start(rwn_sb[L][:, :], rwn[L][:, :])
                ws_sb[L] = wp.tile([64, 128], F32, name=f"ws_sb{L}")
                nc.sync.dma_start(ws_sb[L][:, :], ws[L][:, :])
                wsb_sb[L] = wp.tile([64, 128], F32, name=f"wsb_sb{L}")
                nc.sync.dma_start(wsb_sb[L][:, :], wsb[L][:, :])
            gsT_sb = wp.tile([64, 4], F32)
            nc.sync.dma_start(gsT_sb[:, :], gsT[:, :])
            bsT_sb = wp.tile([64, 4], F32)
            nc.sync.dma_start(bsT_sb[:, :], bsT[:, :])
            nh3 = wp.tile([3, 1], F32)
            nc.vector.memset(nh3[:, :], -0.5)
            nh64 = wp.tile([64, 1], F32)
            nc.vector.memset(nh64[:, :], -0.5)

            # layer-1 aug tensors come pre-assembled from the host
            # ([x; 1; 0] and [x; 0; 1]); the xx rows are filled on device.
            nc.sync.dma_start(aug_l1[:, :], xaugl1[:, :])
            nc.sync.dma_start(aug_r1[:, :], xaugr1[:, :])
            nc.sync.dma_start(aug_l[64:65, :], onesd[:, :])
            nc.sync.dma_start(aug_r[65:66, :], onesd[:, :])

            def layer(L, c, al, ar, nh, stage=99):
                """One DGCNN layer. f (features, (c, n)) must already be in
                al[0:c] and ar[0:c]."""
                # -- xx row: -xx/2 = (-1/2 ones)^T @ f^2 --
                sq = sp.tile([c, n], F32, tag="zw", name=f"sq{L}")
                nc.scalar.activation(sq[:, :], ar[0:c, :], AF.Square)
                xx_sb = sm.tile([1, n], F32, tag="xxsb", bufs=1, name=f"xxsb{L}")
                for t in range(n // ct):
                    xx_ps = mmps.tile([1, ct], F32, tag="mm", name=f"xxps{L}_{t}")
                    nc.tensor.matmul(xx_ps[:, :], lhsT=nh[:, :],
                                     rhs=sq[:, t * ct:(t + 1) * ct])
                    nc.scalar.activation(xx_sb[:, t * ct:(t + 1) * ct],
                                         xx_ps[:, :], AF.Copy)
                nc.sync.dma_start(ar[c:c + 1, :], xx_sb[:, :])
                nc.sync.dma_start(al[c + 1:c + 2, :], xx_sb[:, :])

                # -- z (rows 0:64) and w = z - yc (rows 64:128) in one PSUM tile:
                #    lhsT [lw|lw] puts z on both halves; [0|rwn] adds -yc to
                #    the upper half only.
                zw = sp.tile([128, n], F32, tag="zw", name=f"zw{L}")
                for t in range(n // ct):
                    ts = slice(t * ct, (t + 1) * ct)
                    zw_ps = mmps.tile([128, ct], F32, tag="mm", name=f"zwps{L}_{t}")
                    nc.tensor.matmul(zw_ps[:, :], lhsT=lw_sb[L][:, :], rhs=ar[0:c, ts],
                                     start=True, stop=False)
                    nc.tensor.matmul(zw_ps[:, :], lhsT=rwn_sb[L][:, :], rhs=ar[0:c, ts],
                                     start=False, stop=True)
                    nc.scalar.activation(zw[:, ts], zw_ps[:, :], AF.Copy)
                # wrep (wcol layout, w on both partition halves); DMA APs
                # allow at most 3 [step,count] dims, so split over u
                for h in (0, 1):
                    for u in range(4):
                        dst = wrep[64 * h:64 * h + 64, :].rearrange(
                            "c (u b v) -> c u b v", u=4, v=16)[:, u]
                        srcw = zw[64:128, :].rearrange(
                            "c (b hh u v) -> c hh u b v", hh=2, u=4, v=16)[:, h, u]
                        nc.sync.dma_start(dst, srcw)
                # zrep = [z; z]
                nc.sync.dma_start(zrep[0:64, :], zw[0:64, :])
                nc.sync.dma_start(zrep[64:128, :], zw[0:64, :])

                if stage <= 0:
                    return
                # -- pd blocks + top-k --
                for b in range(nblk):
                    pd = sp.tile([128, n], F32, tag="pd", bufs=2, name=f"pd{L}_{b}")
                    for t in range(n // ct):
                        ts = slice(t * ct, (t + 1) * ct)
                        pd_ps = pdps.tile([128, ct], F32, tag="pd",
                                          name=f"pdps{L}_{b}_{t}")
                        nc.tensor.matmul(pd_ps[:, :],
                                         lhsT=al[:, b * 128:(b + 1) * 128],
                                         rhs=ar[:, ts])
                        nc.scalar.activation(pd[:, ts], pd_ps[:, :], AF.Copy,
                                             scale=2.0)
                    iav = idx_all[:, :].rearrange("p (k bb) -> p k bb", bb=nblk)
                    for r in range(3):
                        m8 = sm.tile([128, 8], F32, tag="m8", name=f"m8_{L}_{b}_{r}")
                        nc.vector.max(m8[:, :], pd[:, :])
                        nc.vector.max_index(iav[:, 8 * r:8 * r + 8, b],
                                            m8[:, :], pd[:, :])
                        if r < 2:
                            nc.vector.match_replace(pd[:, :], in_to_replace=m8[:, :],
                                                    in_values=pd[:, :],
                                                    imm_value=-1e30)

                if stage <= 1:
                    return
                # -- build wrapped gather table --
                for h in (0, 1):
                    for uh in range(4):
                        u = 4 * h + uh
                        srci = idx_all[16 * u:16 * u + 16, :].rearrange(
                            "p (k b) -> p k b", b=nblk)[:, 0:KNN].opt()
                        dsti = idxs[64 * h:64 * h + 16, :].rearrange(
                            "p (k u b) -> p k u b", u=4, k=KNN)[:, :, uh].opt()
                        nc.sync.dma_start(dsti, srci)
                    for g in (1, 2, 3):
                        nc.sync.dma_start(
                            idxs[64 * h + 16 * g:64 * h + 16 * g + 16, :],
                            idxs[64 * h:64 * h + 16, :])

                # -- gather + subtract (y), with free running sum --
                nc.gpsimd.ap_gather(zg[:, :], zrep[:, :], idxs[:, :].bitcast(I16),
                                    channels=128, num_elems=n, d=1, num_idxs=nk2)
                if stage <= 2:
                    return
                w_b = wrep[:, :].unsqueeze(1).broadcast_to((128, KNN, n // 2))
                nc.gpsimd.tensor_tensor(zg[:, :], zg[:, :], w_b, op=ALU.subtract)

                # -- sum / sumsq (ACT accumulators over chunks) --
                sqs = sm.tile([128, nch], F32, name=f"sqs{L}")
                sys_ = sm.tile([128, nch], F32, name=f"sys{L}")
                for cc in range(nch):
                    cs = slice(cc * cw, (cc + 1) * cw)
                    dump = sp.tile([128, cw], F32, tag="chk", name=f"dmp{L}_{cc}")
                    nc.scalar.activation(dump[:, :], zg[:, cs], AF.Square,
                                         accum_out=sqs[:, cc:cc + 1])
                    dump2 = sp.tile([128, cw], F32, tag="chk", name=f"dm2{L}_{cc}")
                    nc.scalar.activation(dump2[:, :], zg[:, cs], AF.Copy,
                                         accum_out=sys_[:, cc:cc + 1])
                sumsq = sm.tile([128, 1], F32, name=f"sumsq{L}")
                nc.vector.reduce_sum(sumsq[:, :], sqs[:, :], axis=mybir.AxisListType.X)
                sumy = sm.tile([128, 1], F32, name=f"sumy{L}")
                nc.vector.reduce_sum(sumy[:, :], sys_[:, :], axis=mybir.AxisListType.X)

                # -- AllReduce of (sum, sumsq) --
                stats = sm.tile([128, 2], F32, name=f"stats{L}")
                nc.vector.tensor_copy(stats[:, 0:1], sumy[:, :])
                nc.vector.tensor_copy(stats[:, 1:2], sumsq[:, :])
                cc_in = dp.tile([128, 2], F32, tag="ccin", bufs=2, name=f"ccin{L}")
                cc_out = dp.tile([128, 2], F32, tag="ccout", bufs=2,
                                 addr_space="Shared", name=f"ccout{L}")
                nc.sync.dma_start(cc_in[:, :], stats[:, :])
                nc.gpsimd.collective_compute("AllReduce", ALU.add, groups,
                                             ins=[cc_in[:, :]], outs=[cc_out[:, :]])
                gst = sm.tile([128, 2], F32, name=f"gst{L}")
                nc.sync.dma_start(gst[:, :], cc_out[:, :])

                # -- finalize scale/bias --
                gstu = sm.tile([64, 2], F32, name=f"gstu{L}")
                nc.sync.dma_start(gstu[:, :], gst[64:128, :])
                ms = sm.tile([64, 2], F32, name=f"ms{L}")
                nc.vector.tensor_tensor(ms[:, :], gst[0:64, :], gstu[:, :],
                                        op=ALU.add)
                nc.vector.tensor_scalar_mul(ms[:, :], ms[:, :], 1.0 / cnt)
                nvar = sm.tile([64, 1], F32, name=f"nvar{L}")
                # nvar = mean*mean - E[y^2];  veps = -nvar + eps = var + eps
                nc.vector.scalar_tensor_tensor(nvar[:, :], ms[:, 0:1], ms[:, 0:1],
                                               ms[:, 1:2], op0=ALU.mult,
                                               op1=ALU.subtract)
                veps = sm.tile([64, 1], F32, name=f"veps{L}")
                nc.vector.tensor_scalar(veps[:, :], nvar[:, :], -1.0, EPS,
                                        op0=ALU.mult, op1=ALU.add)
                rv = sm.tile([64, 1], F32, name=f"rv{L}")
                nc.vector.reciprocal(rv[:, :], veps[:, :])
                rstd = sm.tile([64, 1], F32, name=f"rstd{L}")
                nc.scalar.activation(rstd[:, :], rv[:, :], AF.Sqrt)
                scl = sm.tile([128, 1], F32, name=f"scl{L}")
                nc.vector.tensor_tensor(scl[0:64, :], gsT_sb[:, L - 1:L], rstd[:, :],
                                        op=ALU.mult)
                tmb = sm.tile([64, 1], F32, name=f"tmb{L}")
                nc.vector.tensor_tensor(tmb[:, :], ms[:, 0:1], scl[0:64, :],
                                        op=ALU.mult)
                bia = sm.tile([128, 1], F32, name=f"bia{L}")
                nc.vector.tensor_tensor(bia[0:64, :], bsT_sb[:, L - 1:L], tmb[:, :],
                                        op=ALU.subtract)
                nc.sync.dma_start(scl[64:128, :], scl[0:64, :])
                nc.sync.dma_start(bia[64:128, :], bia[0:64, :])
                ca = (1.0 - ALPHA) / 2.0
                cb = (1.0 + ALPHA) / 2.0
                sa = sm.tile([128, 1], F32, name=f"sa{L}")
                nc.vector.tensor_scalar_mul(sa[:, :], scl[:, :], ca)
                ba = sm.tile([128, 1], F32, name=f"ba{L}")
                nc.vector.tensor_scalar_mul(ba[:, :], bia[:, :], ca)
                sb_ = sm.tile([128, 1], F32, name=f"sb{L}")
                nc.vector.tensor_scalar_mul(sb_[:, :], scl[:, :], cb)
                bb_ = sm.tile([128, 1], F32, name=f"bb{L}")
                nc.vector.tensor_scalar_mul(bb_[:, :], bia[:, :], cb)

                if stage <= 3:
                    return
                # -- normalize + leaky relu, in chunks --
                for cc in range(nch):
                    cs = slice(cc * cw, (cc + 1) * cw)
                    ab = sp.tile([128, cw], F32, tag="chk", name=f"ab{L}_{cc}")
                    nc.scalar.activation(ab[:, :], zg[:, cs], AF.Abs,
                                         bias=ba[:, :], scale=sa[:, :])
                    nc.scalar.activation(zg[:, cs], zg[:, cs], AF.Identity,
                                         bias=bb_[:, :], scale=sb_[:, :])
                    nc.gpsimd.tensor_tensor(zg[:, cs], zg[:, cs], ab[:, :], op=ALU.add)

                if stage <= 4:
                    return
                # -- features for next layer: max over k, then un-shuffle --
                if L < 4 or stage in (41, 42):
                    fw = sp.tile([128, n // 2], F32, tag="fw", bufs=1, name=f"fw{L}")
                    zr = zg[:, :].rearrange("p (k c2) -> p c2 k", k=KNN)
                    nc.vector.reduce_max(fw[:, :], zr, axis=mybir.AxisListType.X)
                    if stage == 41:
                        return
                    for h in (0, 1):
                        for u in range(4):
                            dstf = aug_r[0:64, :].rearrange(
                                "c (b hh u v) -> c hh u b v",
                                hh=2, u=4, v=16)[:, h, u]
                            srcf = fw[64 * h:64 * h + 64, :].rearrange(
                                "c (u b v) -> c u b v", u=4, v=16)[:, u]
                            nc.sync.dma_start(dstf, srcf)
                    nc.scalar.activation(aug_l[0:64, :], aug_r[0:64, :], AF.Copy)
                if stage == 42:
                    return

                # -- skip-path matmul, accumulate in DRAM --
                for cc in range(nch):
                    cs = slice(cc * cw, (cc + 1) * cw)
                    stage = sp.tile([128, cw], F32, tag="chk", name=f"stg{L}_{cc}")
                    # PE accumulation groups need a single base partition;
                    # stage the upper half down to partitions 0:64 via DMA
                    zb = sp.tile([64, cw], F32, tag="chk", name=f"zb{L}_{cc}")
                    nc.sync.dma_start(zb[:, :], zg[64:128, cs])
                    for t in range(cw // ct):
                        ts = slice(cc * cw + t * ct, cc * cw + (t + 1) * ct)
                        ys = ysps.tile([128, ct], F32, tag="ys",
                                       name=f"ys{L}_{cc}_{t}")
                        nc.tensor.matmul(ys[:, :], lhsT=ws_sb[L][:, :],
                                         rhs=zg[0:64, ts],
                                         start=True, stop=False)
                        nc.tensor.matmul(ys[:, :], lhsT=wsb_sb[L][:, :],
                                         rhs=zb[:, t * ct:(t + 1) * ct],
                                         start=False, stop=True)
                        nc.scalar.activation(stage[:, t * ct:(t + 1) * ct],
                                             ys[:, :], AF.Copy)
                    if L == 1:
                        nc.sync.dma_start(acc_dram[:, cs], stage[:, :])
                    else:
                        prev = sp.tile([128, cw], F32, tag="chk",
                                       name=f"prv{L}_{cc}")
                        nc.sync.dma_start(prev[:, :], acc_dram[:, cs])
                        nc.gpsimd.tensor_tensor(stage[:, :], stage[:, :],
                                                prev[:, :], op=ALU.add)
                        if L < 4:
                            nc.sync.dma_start(acc_dram[:, cs], stage[:, :])
                        else:
                            # running max over the chunk's 2 kk-groups
                            hf = cw // 2
                            if cc == 0:
                                nc.vector.tensor_tensor(
                                    out_run[:, :], stage[:, 0:hf],
                                    stage[:, hf:cw], op=ALU.max)
                            else:
                                nc.vector.tensor_tensor(
                                    out_run[:, :], out_run[:, :],
                                    stage[:, 0:hf], op=ALU.max)
                                nc.vector.tensor_tensor(
                                    out_run[:, :], out_run[:, :],
                                    stage[:, hf:cw], op=ALU.max)
                if L == 4:
                    # un-shuffle wcol -> point order into the output
                    for h in (0, 1):
                        for u in range(4):
                            dsto = out_e[:, :].rearrange(
                                "c (b hh u v) -> c hh u b v",
                                hh=2, u=4, v=16)[:, h, u]
                            srco = out_run[64 * h:64 * h + 64, :].rearrange(
                                "c (u b v) -> c u b v", u=4, v=16)[:, u]
                            nc.sync.dma_start(dsto, srco)

            layer(1, 3, aug_l1, aug_r1, nh3, stage=stage)
            if stage >= 6:
                layer(2, 64, aug_l, aug_r, nh64, stage=stage)
            if stage >= 7:
                layer(3, 64, aug_l, aug_r, nh64, stage=stage)
            if stage >= 8:
                layer(4, 64, aug_l, aug_r, nh64, stage=stage)

    nc.compile()
    return nc


_prog_cache = {}


def _get_prog(n=2048):
    if n not in _prog_cache:
        _prog_cache[n] = build_program(n)
    return _prog_cache[n]


def make_in_maps(inputs, n=2048):
    x = np.asarray(inputs["x"], dtype=np.float32)          # (B, n, 3)
    W1 = np.asarray(inputs["W1"], dtype=np.float32)        # (64, 6)
    Wskip = np.asarray(inputs["Wskip"], dtype=np.float32)  # (64, 256)
    shared = {}
    for L, W in ((1, W1), (2, inputs["W2"]), (3, inputs["W3"]), (4, inputs["W4"])):
        W = np.asarray(W, dtype=np.float32)
        c = W.shape[1] // 2
        lwT = W[:, :c].T                      # (c, 64)
        rwT = W[:, c:].T
        shared[f"lw{L}"] = np.ascontiguousarray(
            np.concatenate([lwT, lwT], axis=1))            # (c, 128)
        shared[f"rwn{L}"] = np.ascontiguousarray(
            np.concatenate([np.zeros_like(rwT), -rwT], axis=1))
        wsT = Wskip[:, 64 * (L - 1):64 * L].T              # (64, 64)
        z64 = np.zeros_like(wsT)
        shared[f"wsa{L}"] = np.ascontiguousarray(np.concatenate([wsT, z64], axis=1))
        shared[f"wsb{L}"] = np.ascontiguousarray(np.concatenate([z64, wsT], axis=1))
    shared["gsT"] = np.ascontiguousarray(
        np.stack([np.asarray(inputs[f"g{i}"], dtype=np.float32)
                  for i in range(1, 5)], axis=1))
    shared["bsT"] = np.ascontiguousarray(
        np.stack([np.asarray(inputs[f"b{i}"], dtype=np.float32)
                  for i in range(1, 5)], axis=1))
    n = x.shape[1]
    shared["onesd"] = np.ones((1, n), dtype=np.float32)
    in_maps = []
    for i in range(B):
        m = dict(shared)
        xt = np.ascontiguousarray(x[i].T)  # (3, n)
        m["xT"] = xt
        m["xaugl1"] = np.concatenate(
            [xt, np.ones((1, n), np.float32), np.zeros((1, n), np.float32)], axis=0)
        m["xaugr1"] = np.concatenate(
            [xt, np.zeros((1, n), np.float32), np.ones((1, n), np.float32)], axis=0)
        in_maps.append(m)
    return in_maps


def kernel(**inputs):
    n = np.asarray(inputs["x"]).shape[1]
    nc = _get_prog(n)
    in_maps = make_in_maps(inputs, n)
    res = run_bass_kernel_spmd(nc, in_maps, core_ids=list(range(B)))
    return np.stack([r["out"] for r in res.results]).astype(np.float32)
